# revision 9
# baseline (speedup 1.0000x reference)
"""Causal self-attention (GQA + RoPE) for Trainium2, 8 NeuronCores.

Sharding: core c handles batch b = c // 4 and kv-group g = c % 4
(4 q-heads + 1 kv-head per core).  Each core computes its heads'
attention output and a row-parallel partial of the output projection;
the host sums the 4 partials per batch.

Fused single-pipeline schedule (v2):
  - xT and all weights are SBUF-resident; projections re-read x from
    SBUF so PSUM needs only 2 banks (single-tile passes, double
    buffered).
  - V is projected directly into [s, d] layout (stationary = x block,
    moving = wv), eliminating the PE transpose + DVE copies.
  - Work is emitted chunk-by-chunk: proj(ch) then attention(qc=ch),
    with proj(ch+1) and the previous chunk's output projection
    interleaved one matmul at a time between the ACT-gated
    scores/exp/PV steps so the PE never starves.
  - The scores->exp->P@V inner loop is software-pipelined (scores
    issue LAG steps ahead of the matching P@V) so the in-order PE
    never waits on the exp() latency.
  - Softmax denominator: P tiles are accumulated on DVE (4x mode),
    partition-reduced on GPSIMD (partition_all_reduce), inverted with
    the fast DVE reciprocal; no PE ones-matmul and no PSUM bank.
  - Output projection accumulates in PSUM (sharing the score-tile PSUM
    ring), is evicted bf16 by DVE/ACT alternately, and streams out by
    DMA; y partials are bf16 and the host sums them in fp32.
  - Scores are computed transposed (ST[k, q] = K_blk Q^T) so P^T feeds
    P@V with no transpose; exp uses a constant bias (no row max,
    |scores| < 5) which cancels in the normalization; Q/K head dims
    are de-interleaved so RoPE's rotate-half is a +-64-partition swap
    (wq/wk columns permuted on the host).
"""

import os
import sys

import numpy as np

for _p in ("/opt/trn_rl_repo", os.path.expanduser("~/.axon_site/_ro/trn_rl_repo")):
    if os.path.isdir(_p) and _p not in sys.path:
        sys.path.append(_p)

B, S, D = 2, 2048, 2048
NH_TOT, NKV, HD = 16, 4, 128
N_CORES = 8
NHC = NH_TOT // NKV          # q heads per core = 4
DQ = NHC * HD                # 512
NB = S // 128                # 16 c-blocks of the contraction dim
CH = 512                     # free-dim chunk (one fp32 PSUM bank)
NCH = S // CH                # 4
SCALE = HD ** -0.5
EXP_BIAS = -4.0
ROPE_THETA = 10000.0

_CACHE = {}


def _build_nc(repeat=1):
    """Build the SPMD program; repeat>1 duplicates the whole computation
    in one NEFF (used only to measure device time via the wall-clock
    slope between repeat counts)."""
    import concourse.mybir as mybir
    import concourse.tile as tile
    from concourse import bacc
    from concourse import bass_isa

    F32 = mybir.dt.float32
    BF = mybir.dt.bfloat16
    EXP = mybir.ActivationFunctionType.Exp

    nc = bacc.Bacc(None, target_bir_lowering=False)

    # all inputs come pre-arranged from the host in the exact SBUF
    # layout (partition-major) so every DMA is contiguous per partition
    # line — rearranging DMAs would be descriptor-bound (256B/desc)
    xT = nc.declare_dram_parameter("xT", [128, NCH, NB, CH], BF, isOutput=False)
    wq = nc.declare_dram_parameter("wq", [128, NHC, NB, HD], BF, isOutput=False)
    wk = nc.declare_dram_parameter("wk", [128, NB, HD], BF, isOutput=False)
    wv = nc.declare_dram_parameter("wv", [128, NB, HD], BF, isOutput=False)
    wo = nc.declare_dram_parameter("wo", [128, NHC, D], BF, isOutput=False)
    # cosF[j] = cos(freq_{j%64}); sinF[j<64] = -sin, sinF[j>=64] = +sin so
    # rotate-half reduces to dst = src*cosF + swapped(src)*sinF
    cosT = nc.declare_dram_parameter("cosT", [128, S], BF, isOutput=False)
    sinT = nc.declare_dram_parameter("sinT", [128, S], BF, isOutput=False)
    msk = nc.declare_dram_parameter("msk", [128, 128], BF, isOutput=False)
    yout = nc.declare_dram_parameter("y", [S, D], BF, isOutput=True)

    def _one(tc):
        with (
            tc.tile_pool(name="const", bufs=1) as const,
            tc.tile_pool(name="pers", bufs=1) as pers,
            tc.tile_pool(name="otp", bufs=9) as otp,
            tc.tile_pool(name="sbw", bufs=3) as sbw,
            tc.tile_pool(name="pjp", bufs=2, space="PSUM") as pjp,
            tc.tile_pool(name="stp", bufs=4, space="PSUM") as stp,
            tc.tile_pool(name="otps", bufs=2, space="PSUM") as otps,
        ):
            bias_t = const.tile([128, 1], F32)
            nc.any.memset(bias_t[:], EXP_BIAS)
            tri_sb = const.tile([128, 128], BF)

            xsb = pers.tile([128, NCH, NB, CH], BF)
            wq_sb = pers.tile([128, NHC, NB, HD], BF)
            wk_sb = pers.tile([128, NB, HD], BF)
            wv_sb = pers.tile([128, NB, HD], BF)
            wo_sb = pers.tile([128, NHC, D], BF)
            cos_sb = pers.tile([128, S], BF)
            sin_sb = pers.tile([128, S], BF)
            kt = pers.tile([128, S], BF)
            vsb = pers.tile([128, NB, HD], BF)
            qt = [
                pers.tile([128, S], BF, tag=f"qt{h}", name=f"qt{h}")
                for h in range(NHC)
            ]

            # ---------- prologue DMAs ----------
            # The DMA engines are effectively one serial ~700GB/s
            # resource, so transfers are queued on ONE queue (SP) in the
            # exact order compute needs them: x chunk 0 first
            # (interleaved with the weights), cos/sin early for RoPE, wo
            # and the remaining x chunks last (not needed until the
            # first attention segment).
            def _x_dma(cb, n, ch):
                nc.sync.dma_start(
                    xsb[:, ch, cb : cb + n, :], xT[:, ch, cb : cb + n, :]
                )

            _x_dma(0, 1, 0)
            nc.sync.dma_start(wk_sb[:, 0:1, :], wk[:, 0:1, :])
            _x_dma(1, 1, 0)
            nc.sync.dma_start(wk_sb[:, 1:, :], wk[:, 1:, :])
            nc.sync.dma_start(wv_sb[:], wv[:])
            _x_dma(2, 2, 0)
            _x_dma(4, 2, 0)
            _x_dma(6, 2, 0)
            _x_dma(8, 2, 0)
            nc.sync.dma_start(wq_sb[:, 0], wq[:, 0])
            _x_dma(10, 2, 0)
            _x_dma(12, 2, 0)
            _x_dma(14, 2, 0)
            for h in range(1, NHC):
                nc.sync.dma_start(wq_sb[:, h], wq[:, h])
            nc.sync.dma_start(cos_sb[:], cosT[:])
            nc.sync.dma_start(sin_sb[:], sinT[:])
            nc.sync.dma_start(tri_sb[:], msk[:])
            # remaining x chunks + wo stream in the background
            for ch in range(1, NCH):
                _x_dma(0, NB, ch)
            nc.sync.dma_start(wo_sb[:], wo[:])

            def rope_evict(ps, dst, sl):
                # De-interleaved layout: partition j<64 holds orig dim 2j,
                # partition j+64 holds orig dim 2j+1.  ACT evicts PSUM
                # (fp32), then DVE runs at SBUF speed; sinF's sign fold
                # makes the final combine a single full-width add.
                # (walrus requires equal base partitions when BOTH
                # TensorTensor inputs are SBUF, so the half-swap is done
                # with single-input DVE copies, which are exempt)
                sc = sbw.tile([128, CH], BF, tag="ropesc")
                nc.scalar.copy(sc[:], ps[:])
                scw = sbw.tile([128, CH], BF, tag="ropescw")
                nc.vector.tensor_copy(scw[0:64, :], sc[64:128, :])
                nc.vector.tensor_copy(scw[64:128, :], sc[0:64, :])
                tmp = sbw.tile([128, CH], BF, tag="ropetmp")
                tmp2 = sbw.tile([128, CH], BF, tag="ropetmp2")
                nc.vector.tensor_mul(tmp[:], scw[:], sin_sb[:, sl])
                nc.vector.tensor_mul(tmp2[:], sc[:], cos_sb[:, sl])
                nc.vector.tensor_add(dst[:, sl], tmp2[:], tmp[:])

            def proj_chunk_gen(ch):
                """Emit projection of chunk ch; yields ~once per 512 PE rows.
                Pass order k, v, q0..q3: the small-weight passes run while
                wq streams in (chunk 0 is DMA-paced at the start)."""
                sl = slice(ch * CH, (ch + 1) * CH)
                # K and V interleaved per c-block: chunk 0 is paced by
                # the x DMA stream, and one k-matmul + four skinny
                # v-matmuls consume a c-block at about the DMA rate.
                # V is computed directly in [s, d] layout: stationary =
                # x block, moving = wv; out[s, d] accumulates over c.
                # The four 128-row s-blocks share one PSUM bank; the
                # j=0,c=0 start=True zeroes the whole 2KB region so j>0
                # chains accumulate onto pending-zero (skip the group
                # check).
                psk = pjp.tile([128, CH], F32, tag="pj", name=f"pjk{ch}")
                psv = pjp.tile([128, CH], F32, tag="pj", name=f"pjv{ch}")
                for c in range(NB):
                    nc.tensor.matmul(
                        psk[:], wk_sb[:, c, :], xsb[:, ch, c, :],
                        start=(c == 0), stop=(c == NB - 1),
                    )
                    yield
                    for j in range(4):
                        nc.tensor.matmul(
                            psv[:, j * 128 : (j + 1) * 128],
                            xsb[:, ch, c, j * 128 : (j + 1) * 128],
                            wv_sb[:, c, :],
                            start=(j == 0 and c == 0),
                            stop=(c == NB - 1),
                            skip_group_check=(j > 0),
                        )
                    yield
                rope_evict(psk, kt, sl)
                nc.scalar.copy(vsb[:, ch * 4 : (ch + 1) * 4, :], psv[:])
                for h in range(NHC):
                    ps = pjp.tile([128, CH], F32, tag="pj", name=f"pjq{ch}_{h}")
                    for c in range(NB):
                        nc.tensor.matmul(
                            ps[:], wq_sb[:, h, c, :], xsb[:, ch, c, :],
                            start=(c == 0), stop=(c == NB - 1),
                        )
                        yield
                    rope_evict(ps, qt[h], sl)

            def wo_gen(qc, ot_sb):
                """Emit output projection for q-chunk qc; yields per matmul.
                y accumulates in PSUM (st ring), DVE/ACT evict to SBUF (DMA
                from PSUM is not supported), DMA streams it out."""
                for sb_i in range(4):
                    srow = qc * 4 + sb_i
                    for ec in range(NCH):
                        y_ps = stp.tile([128, CH], F32, tag="st",
                                        name=f"y{qc}_{sb_i}_{ec}")
                        for h in range(NHC):
                            nc.tensor.matmul(
                                y_ps[:],
                                ot_sb[h][:, sb_i * 128 : (sb_i + 1) * 128],
                                wo_sb[:, h, ec * CH : (ec + 1) * CH],
                                start=(h == 0), stop=(h == NHC - 1),
                            )
                            yield
                        ysb = sbw.tile([128, CH], BF, tag="ysb", bufs=4)
                        # alternate the evicting engine so neither ACT nor
                        # DVE becomes the drain bottleneck
                        if ec % 2 == 0:
                            nc.vector.tensor_copy(ysb[:], y_ps[:])
                        else:
                            nc.scalar.copy(ysb[:], y_ps[:])
                        nc.sync.dma_start(
                            yout[
                                srow * 128 : (srow + 1) * 128,
                                ec * CH : (ec + 1) * CH,
                            ],
                            ysb[:],
                        )

            def _chain(*gens):
                for g in gens:
                    yield from g

            def _pull(gen, n):
                for _ in range(n):
                    if next(gen, None) is None:
                        return

            # ---------- fused pipeline ----------
            for _ in proj_chunk_gen(0):
                pass
            prev_ot = None
            for qc in range(NCH):
                filler = _chain(
                    *( [proj_chunk_gen(qc + 1)] if qc + 1 < NCH else [] ),
                    *( [wo_gen(qc - 1, prev_ot)] if qc > 0 else [] ),
                )
                kbmax = 4 * qc + 4
                LAG = 2  # software pipeline: scores issue LAG steps
                # ahead of the matching P@V so the in-order PE never
                # sits on the exp() latency
                ot_sb = []
                for h in range(NHC):
                    ot_ps = otps.tile([128, CH], F32, tag="ot")
                    acc = sbw.tile([128, CH], BF, tag="acc", bufs=2)
                    pts = {}

                    def _c0(kb):
                        return max(kb - 4 * qc, 0) * 128

                    for step in range(kbmax + LAG):
                        if step < kbmax:
                            kb, c0 = step, _c0(step)
                            st_t = stp.tile([128, CH], F32, tag="st")
                            nc.tensor.matmul(
                                st_t[:, c0:],
                                kt[:, kb * 128 : (kb + 1) * 128],
                                qt[h][:, qc * CH + c0 : (qc + 1) * CH],
                                start=True,
                                stop=True,
                            )
                            pt = sbw.tile([128, CH], BF, tag="pt", bufs=10)
                            nc.scalar.activation(
                                pt[:, c0:], st_t[:, c0:], EXP,
                                bias=bias_t[:], scale=SCALE,
                            )
                            if kb - 4 * qc >= 0:  # diagonal block
                                nc.vector.tensor_mul(
                                    pt[:, c0 : c0 + 128],
                                    pt[:, c0 : c0 + 128],
                                    tri_sb[:],
                                )
                            pts[kb] = pt
                            _pull(filler, 1)
                        if step >= LAG:
                            kb, c0 = step - LAG, _c0(step - LAG)
                            pt = pts.pop(kb)
                            # columns < c0 are never read: P@V and the
                            # acc accumulation are narrowed to [c0:]
                            nc.tensor.matmul(
                                ot_ps[:, c0:], vsb[:, kb, :], pt[:, c0:],
                                start=(kb == 0), stop=(kb == kbmax - 1),
                            )
                            _pull(filler, 1)
                            if kb == 0:
                                nc.vector.tensor_copy(acc[:], pt[:])
                            else:
                                nc.vector.tensor_add(
                                    acc[:, c0:], acc[:, c0:], pt[:, c0:]
                                )
                    lsum = sbw.tile([128, CH], F32, tag="lsum", bufs=2)
                    nc.gpsimd.partition_all_reduce(
                        lsum[:], acc[:], 128, bass_isa.ReduceOp.add
                    )
                    rlb = sbw.tile([128, CH], F32, tag="rlb", bufs=2)
                    nc.vector.reciprocal_approx_fast(rlb[:], lsum[:])
                    o = otp.tile([128, CH], BF, tag="otsb", name=f"o{qc}_{h}")
                    nc.vector.tensor_mul(o[:], ot_ps[:], rlb[:])
                    ot_sb.append(o)
                    _pull(filler, 2)
                # drain leftover proj/wo work for this segment
                for _ in filler:
                    pass
                prev_ot = ot_sb
            for _ in wo_gen(NCH - 1, prev_ot):
                pass

    with tile.TileContext(nc) as tc:
        for _rep in range(repeat):
            _one(tc)

    nc.compile()
    return nc


def _rope_tables():
    inv = 1.0 / (
        np.float32(ROPE_THETA)
        ** (np.arange(0, HD, 2, dtype=np.float32) / np.float32(HD))
    )
    pos = np.arange(S, dtype=np.float32)
    freqs = np.outer(pos, inv).astype(np.float32)  # [S, 64]
    import ml_dtypes

    c = np.cos(freqs).T  # [64, S]
    s = np.sin(freqs).T
    cosF = np.concatenate([c, c], axis=0)            # [128, S]
    sinF = np.concatenate([-s, s], axis=0)           # sign-folded
    bf16 = ml_dtypes.bfloat16
    return (
        np.ascontiguousarray(cosF).astype(bf16),
        np.ascontiguousarray(sinF).astype(bf16),
    )


def _prep_in_maps(x, wq, wk, wv, wo):
    import ml_dtypes

    bf16 = ml_dtypes.bfloat16
    x = np.asarray(x, dtype=np.float32)
    wq = np.asarray(wq, dtype=np.float32).astype(bf16)
    wk = np.asarray(wk, dtype=np.float32).astype(bf16)
    wv = np.asarray(wv, dtype=np.float32).astype(bf16)
    wo = np.asarray(wo, dtype=np.float32).astype(bf16)

    perm = np.concatenate([np.arange(0, HD, 2), np.arange(1, HD, 2)])
    cosT, sinT = _rope_tables()

    # tri[k, j] = 1 where j >= k (within-block causal triangle)
    msk = (
        np.arange(128)[None, :] >= np.arange(128)[:, None]
    ).astype(bf16)

    def _pmaj(w, nblk):
        # [nblk*128, M] -> [128, nblk, M] partition-major SBUF layout
        return np.ascontiguousarray(
            w.reshape(nblk, 128, w.shape[-1]).transpose(1, 0, 2)
        )

    in_maps = []
    for c in range(N_CORES):
        b, g = divmod(c, NKV)
        qcols = np.concatenate([(g * NHC + h) * HD + perm for h in range(NHC)])
        xTc = x[b].T.astype(bf16)  # [D, S]
        # xP[p, ch, cblk, s'] = xT[cblk*128+p, ch*CH+s']
        xP = np.ascontiguousarray(
            xTc.reshape(NB, 128, NCH, CH).transpose(1, 2, 0, 3)
        )
        # wq head-major: [128, NHC, NB, HD] so each head's slice is one
        # contiguous DMA
        wqp = _pmaj(wq[:, qcols], NB)  # [128, NB, NHC*HD]
        wqp = np.ascontiguousarray(
            wqp.reshape(128, NB, NHC, HD).transpose(0, 2, 1, 3)
        )
        in_maps.append(
            {
                "xT": xP,
                "wq": wqp,
                "wk": _pmaj(wk[:, g * HD + perm], NB),
                "wv": _pmaj(wv[:, g * HD : (g + 1) * HD], NB),
                "wo": _pmaj(wo[g * DQ : (g + 1) * DQ, :], NHC),
                "cosT": cosT,
                "sinT": sinT,
                "msk": msk,
            }
        )
    return in_maps


class _Runner:
    """Build + jit the SPMD program once; reuse across kernel() calls.

    Mirrors bass_utils.run_bass_kernel_spmd's axon path (bass2jax
    run_bass_via_pjrt), but caches the jitted executable so repeated
    kernel() calls don't re-run the multi-minute NEFF compile.
    """

    def __init__(self, repeat=1):
        import jax
        import concourse.mybir as mybir
        from concourse import bass2jax
        from jax.experimental.shard_map import shard_map
        from jax.sharding import Mesh, PartitionSpec

        self.jax = jax
        nc = _build_nc(repeat)
        assert nc.dbg_addr is None
        bass2jax.install_neuronx_cc_hook()

        partition_name = (
            nc.partition_id_tensor.name if nc.partition_id_tensor else None
        )
        in_names, out_names, out_avals, zero_outs = [], [], [], []
        for alloc in nc.m.functions[0].allocations:
            if not isinstance(alloc, mybir.MemoryLocationSet):
                continue
            name = alloc.memorylocations[0].name
            if alloc.kind == "ExternalInput":
                if name != partition_name:
                    in_names.append(name)
            elif alloc.kind == "ExternalOutput":
                shape = tuple(alloc.tensor_shape)
                dtype = mybir.dt.np(alloc.dtype)
                out_names.append(name)
                out_avals.append(jax.core.ShapedArray(shape, dtype))
                zero_outs.append(np.zeros(shape, dtype))
        n_params = len(in_names)
        all_in = in_names + out_names + (
            [partition_name] if partition_name else []
        )

        def _body(*args):
            operands = list(args)
            if partition_name is not None:
                operands.append(bass2jax.partition_id_tensor())
            outs = bass2jax._bass_exec_p.bind(
                *operands,
                out_avals=tuple(out_avals),
                in_names=tuple(all_in),
                out_names=tuple(out_names),
                lowering_input_output_aliases=(),
                sim_require_finite=True,
                sim_require_nnan=True,
                nc=nc,
            )
            return tuple(outs)

        devices = jax.devices()[:N_CORES]
        assert len(devices) == N_CORES
        self.mesh = Mesh(np.asarray(devices), ("core",))
        in_specs = (PartitionSpec("core"),) * (n_params + len(out_names))
        out_specs = (PartitionSpec("core"),) * len(out_names)
        donate = tuple(range(n_params, n_params + len(out_names)))
        self.fn = jax.jit(
            shard_map(
                _body,
                mesh=self.mesh,
                in_specs=in_specs,
                out_specs=out_specs,
                check_rep=False,
            ),
            donate_argnums=donate,
            keep_unused=True,
        )
        self.in_names = in_names
        self.out_names = out_names
        self.out_avals = out_avals

    def concat_inputs(self, in_maps):
        return [
            np.concatenate([np.asarray(m[name]) for m in in_maps], axis=0)
            for name in self.in_names
        ]

    def zero_outputs(self):
        return [
            np.zeros((N_CORES * a.shape[0], *a.shape[1:]), a.dtype)
            for a in self.out_avals
        ]

    def time_iters(self, in_maps, iters=20, warmup=3):
        """Amortized per-call wall time (s) with device-resident inputs.

        Reuses each call's outputs as the next call's donated output
        buffers (the kernel writes every output element, so their
        contents don't matter).
        """
        import time

        jax = self.jax
        from jax.sharding import NamedSharding, PartitionSpec

        sh = NamedSharding(self.mesh, PartitionSpec("core"))
        ins = [jax.device_put(a, sh) for a in self.concat_inputs(in_maps)]
        outs = self.fn(
            *ins, *[jax.device_put(z, sh) for z in self.zero_outputs()]
        )
        for _ in range(warmup):
            outs = self.fn(*ins, *outs)
        jax.block_until_ready(outs)
        t0 = time.perf_counter()
        for _ in range(iters):
            outs = self.fn(*ins, *outs)
        jax.block_until_ready(outs)
        return (time.perf_counter() - t0) / iters

    def run(self, in_maps):
        out_arrs = self.fn(*self.concat_inputs(in_maps), *self.zero_outputs())
        return [
            {
                name: np.asarray(out_arrs[i]).reshape(
                    N_CORES, *self.out_avals[i].shape
                )[c]
                for i, name in enumerate(self.out_names)
            }
            for c in range(N_CORES)
        ]


def _get_runner():
    if "runner" not in _CACHE:
        _CACHE["runner"] = _Runner()
    return _CACHE["runner"]


def kernel(x, wq, wk, wv, wo):
    runner = _get_runner()
    results = runner.run(_prep_in_maps(x, wq, wk, wv, wo))
    y = np.zeros((B, S, D), dtype=np.float32)
    for c in range(N_CORES):
        y[c // NKV] += results[c]["y"].astype(np.float32)
    return y


# revision 10
# speedup vs baseline: 1.1779x; 1.1779x over previous
"""Causal self-attention (GQA + RoPE) for Trainium2, 8 NeuronCores.

Sharding: core c handles batch b = c // 4 and kv-group g = c % 4
(4 q-heads + 1 kv-head per core).  Each core computes its heads'
attention output and a row-parallel partial of the output projection;
the host sums the 4 partials per batch.

Fused single-pipeline schedule (v2):
  - xT and all weights are SBUF-resident; projections re-read x from
    SBUF so PSUM needs only 2 banks (single-tile passes, double
    buffered).
  - V is projected directly into [s, d] layout (stationary = x block,
    moving = wv), eliminating the PE transpose + DVE copies.
  - Work is emitted chunk-by-chunk: proj(ch) then attention(qc=ch),
    with proj(ch+1) and the previous chunk's output projection
    interleaved one matmul at a time between the ACT-gated
    scores/exp/PV steps so the PE never starves.
  - The scores->exp->P@V inner loop is software-pipelined (scores
    issue LAG steps ahead of the matching P@V) so the in-order PE
    never waits on the exp() latency.
  - Softmax denominator: P tiles are accumulated on DVE (4x mode),
    partition-reduced on GPSIMD (partition_all_reduce), inverted with
    the fast DVE reciprocal; no PE ones-matmul and no PSUM bank.
  - Output projection accumulates in PSUM (sharing the score-tile PSUM
    ring), is evicted bf16 by DVE/ACT alternately, and streams out by
    DMA; y partials are bf16 and the host sums them in fp32.
  - Scores are computed transposed (ST[k, q] = K_blk Q^T) so P^T feeds
    P@V with no transpose; exp uses a constant bias (no row max,
    |scores| < 5) which cancels in the normalization; Q/K head dims
    are de-interleaved so RoPE's rotate-half is a +-64-partition swap
    (wq/wk columns permuted on the host).
"""

import os
import sys

import numpy as np

for _p in ("/opt/trn_rl_repo", os.path.expanduser("~/.axon_site/_ro/trn_rl_repo")):
    if os.path.isdir(_p) and _p not in sys.path:
        sys.path.append(_p)

B, S, D = 2, 2048, 2048
NH_TOT, NKV, HD = 16, 4, 128
N_CORES = 8
NHC = NH_TOT // NKV          # q heads per core = 4
DQ = NHC * HD                # 512
NB = S // 128                # 16 c-blocks of the contraction dim
CH = 512                     # free-dim chunk (one fp32 PSUM bank)
NCH = S // CH                # 4
SCALE = HD ** -0.5
EXP_BIAS = -4.0
ROPE_THETA = 10000.0

_CACHE = {}


def _build_nc(repeat=1):
    """Build the SPMD program; repeat>1 duplicates the whole computation
    in one NEFF (used only to measure device time via the wall-clock
    slope between repeat counts)."""
    import concourse.mybir as mybir
    import concourse.tile as tile
    from concourse import bacc
    from concourse import bass_isa

    F32 = mybir.dt.float32
    BF = mybir.dt.bfloat16
    EXP = mybir.ActivationFunctionType.Exp

    nc = bacc.Bacc(None, target_bir_lowering=False)

    # all inputs come pre-arranged from the host in the exact SBUF
    # layout (partition-major) so every DMA is contiguous per partition
    # line — rearranging DMAs would be descriptor-bound (256B/desc)
    xT = nc.declare_dram_parameter("xT", [128, NCH, NB, CH], BF, isOutput=False)
    wq = nc.declare_dram_parameter("wq", [128, NHC, NB, HD], BF, isOutput=False)
    wk = nc.declare_dram_parameter("wk", [128, NB, HD], BF, isOutput=False)
    wv = nc.declare_dram_parameter("wv", [128, NB, HD], BF, isOutput=False)
    wo = nc.declare_dram_parameter("wo", [128, NHC, D], BF, isOutput=False)
    # cosF[j] = cos(freq_{j%64}); sinF[j<64] = -sin, sinF[j>=64] = +sin so
    # rotate-half reduces to dst = src*cosF + swapped(src)*sinF
    cosT = nc.declare_dram_parameter("cosT", [128, S], BF, isOutput=False)
    sinT = nc.declare_dram_parameter("sinT", [128, S], BF, isOutput=False)
    msk = nc.declare_dram_parameter("msk", [128, 128], BF, isOutput=False)
    yout = nc.declare_dram_parameter("y", [S, D], BF, isOutput=True)

    def _one(tc):
        with (
            tc.tile_pool(name="const", bufs=1) as const,
            tc.tile_pool(name="pers", bufs=1) as pers,
            tc.tile_pool(name="otp", bufs=9) as otp,
            tc.tile_pool(name="sbw", bufs=3) as sbw,
            tc.tile_pool(name="pjp", bufs=2, space="PSUM") as pjp,
            tc.tile_pool(name="stp", bufs=4, space="PSUM") as stp,
            tc.tile_pool(name="otps", bufs=2, space="PSUM") as otps,
        ):
            bias_t = const.tile([128, 1], F32)
            nc.any.memset(bias_t[:], EXP_BIAS)
            tri_sb = const.tile([128, 128], BF)

            xsb = pers.tile([128, NCH, NB, CH], BF)
            wq_sb = pers.tile([128, NHC, NB, HD], BF)
            wk_sb = pers.tile([128, NB, HD], BF)
            wv_sb = pers.tile([128, NB, HD], BF)
            wo_sb = pers.tile([128, NHC, D], BF)
            cos_sb = pers.tile([128, S], BF)
            sin_sb = pers.tile([128, S], BF)
            kt = pers.tile([128, S], BF)
            vsb = pers.tile([128, NB, HD], BF)
            qt = [
                pers.tile([128, S], BF, tag=f"qt{h}", name=f"qt{h}")
                for h in range(NHC)
            ]

            # ---------- prologue DMAs ----------
            # The DMA engines are effectively one serial ~700GB/s
            # resource, so transfers are queued on ONE queue (SP) in the
            # exact order compute needs them: x chunk 0 first
            # (interleaved with the weights), cos/sin early for RoPE, wo
            # and the remaining x chunks last (not needed until the
            # first attention segment).
            def _x_dma(cb, n, ch):
                nc.sync.dma_start(
                    xsb[:, ch, cb : cb + n, :], xT[:, ch, cb : cb + n, :]
                )

            _x_dma(0, 1, 0)
            nc.sync.dma_start(wk_sb[:, 0:2, :], wk[:, 0:2, :])
            _x_dma(1, 1, 0)
            nc.sync.dma_start(wv_sb[:], wv[:])
            nc.sync.dma_start(wk_sb[:, 2:, :], wk[:, 2:, :])
            _x_dma(2, 2, 0)
            _x_dma(4, 2, 0)
            _x_dma(6, 2, 0)
            _x_dma(8, 2, 0)
            nc.sync.dma_start(wq_sb[:, 0], wq[:, 0])
            _x_dma(10, 2, 0)
            _x_dma(12, 2, 0)
            _x_dma(14, 2, 0)
            for h in range(1, NHC):
                nc.sync.dma_start(wq_sb[:, h], wq[:, h])
            nc.sync.dma_start(cos_sb[:], cosT[:])
            nc.sync.dma_start(sin_sb[:], sinT[:])
            nc.sync.dma_start(tri_sb[:], msk[:])
            # remaining x chunks + wo stream in the background
            for ch in range(1, NCH):
                _x_dma(0, NB, ch)
            nc.sync.dma_start(wo_sb[:], wo[:])

            def rope_evict(ps, dst, sl):
                # De-interleaved layout: partition j<64 holds orig dim 2j,
                # partition j+64 holds orig dim 2j+1.  ACT evicts PSUM
                # (fp32), then DVE runs at SBUF speed; sinF's sign fold
                # makes the final combine a single full-width add.
                # (walrus requires equal base partitions when BOTH
                # TensorTensor inputs are SBUF, so the half-swap is done
                # with single-input DVE copies, which are exempt)
                sc = sbw.tile([128, CH], BF, tag="ropesc")
                nc.scalar.copy(sc[:], ps[:])
                scw = sbw.tile([128, CH], BF, tag="ropescw")
                nc.vector.tensor_copy(scw[0:64, :], sc[64:128, :])
                nc.vector.tensor_copy(scw[64:128, :], sc[0:64, :])
                tmp = sbw.tile([128, CH], BF, tag="ropetmp")
                tmp2 = sbw.tile([128, CH], BF, tag="ropetmp2")
                nc.vector.tensor_mul(tmp[:], scw[:], sin_sb[:, sl])
                nc.vector.tensor_mul(tmp2[:], sc[:], cos_sb[:, sl])
                nc.vector.tensor_add(dst[:, sl], tmp2[:], tmp[:])

            def proj_chunk_gen(ch):
                """Emit projection of chunk ch; yields ~once per 512 PE rows.
                Pass order k, v, q0..q3: the small-weight passes run while
                wq streams in (chunk 0 is DMA-paced at the start)."""
                sl = slice(ch * CH, (ch + 1) * CH)
                # K and V interleaved per c-block: chunk 0 is paced by
                # the x DMA stream, and one k-matmul + four skinny
                # v-matmuls consume a c-block at about the DMA rate.
                # V is computed directly in [s, d] layout: stationary =
                # x block, moving = wv; out[s, d] accumulates over c.
                # The four 128-row s-blocks share one PSUM bank; the
                # j=0,c=0 start=True zeroes the whole 2KB region so j>0
                # chains accumulate onto pending-zero (skip the group
                # check).
                psk = pjp.tile([128, CH], F32, tag="pj", name=f"pjk{ch}")
                psv = pjp.tile([128, CH], F32, tag="pj", name=f"pjv{ch}")
                for c in range(NB):
                    nc.tensor.matmul(
                        psk[:], wk_sb[:, c, :], xsb[:, ch, c, :],
                        start=(c == 0), stop=(c == NB - 1),
                    )
                    yield
                    for j in range(4):
                        nc.tensor.matmul(
                            psv[:, j * 128 : (j + 1) * 128],
                            xsb[:, ch, c, j * 128 : (j + 1) * 128],
                            wv_sb[:, c, :],
                            start=(j == 0 and c == 0),
                            stop=(c == NB - 1),
                            skip_group_check=(j > 0),
                        )
                    yield
                rope_evict(psk, kt, sl)
                nc.scalar.copy(vsb[:, ch * 4 : (ch + 1) * 4, :], psv[:])
                for h in range(NHC):
                    ps = pjp.tile([128, CH], F32, tag="pj", name=f"pjq{ch}_{h}")
                    for c in range(NB):
                        nc.tensor.matmul(
                            ps[:], wq_sb[:, h, c, :], xsb[:, ch, c, :],
                            start=(c == 0), stop=(c == NB - 1),
                        )
                        yield
                    rope_evict(ps, qt[h], sl)

            def wo_gen(qc, ot_sb):
                """Emit output projection for q-chunk qc; yields per matmul.
                y accumulates in PSUM (st ring), DVE/ACT evict to SBUF (DMA
                from PSUM is not supported), DMA streams it out."""
                for sb_i in range(4):
                    srow = qc * 4 + sb_i
                    for ec in range(NCH):
                        y_ps = stp.tile([128, CH], F32, tag="st",
                                        name=f"y{qc}_{sb_i}_{ec}")
                        for h in range(NHC):
                            nc.tensor.matmul(
                                y_ps[:],
                                ot_sb[h][:, sb_i * 128 : (sb_i + 1) * 128],
                                wo_sb[:, h, ec * CH : (ec + 1) * CH],
                                start=(h == 0), stop=(h == NHC - 1),
                            )
                            yield
                        ysb = sbw.tile([128, CH], BF, tag="ysb", bufs=4)
                        # alternate the evicting engine so neither ACT nor
                        # DVE becomes the drain bottleneck
                        if ec % 2 == 0:
                            nc.vector.tensor_copy(ysb[:], y_ps[:])
                        else:
                            nc.scalar.copy(ysb[:], y_ps[:])
                        nc.sync.dma_start(
                            yout[
                                srow * 128 : (srow + 1) * 128,
                                ec * CH : (ec + 1) * CH,
                            ],
                            ysb[:],
                        )

            def _chain(*gens):
                for g in gens:
                    yield from g

            def _pull(gen, n):
                for _ in range(n):
                    if next(gen, None) is None:
                        return

            # ---------- fused pipeline ----------
            for _ in proj_chunk_gen(0):
                pass
            prev_ot = None
            for qc in range(NCH):
                filler = _chain(
                    *( [proj_chunk_gen(qc + 1)] if qc + 1 < NCH else [] ),
                    *( [wo_gen(qc - 1, prev_ot)] if qc > 0 else [] ),
                )
                kbmax = 4 * qc + 4
                LAG = 2  # software pipeline: scores issue LAG steps
                # ahead of the matching P@V so the in-order PE never
                # sits on the exp() latency
                ot_sb = []
                for h in range(NHC):
                    ot_ps = otps.tile([128, CH], F32, tag="ot")
                    acc = sbw.tile([128, CH], BF, tag="acc", bufs=2)
                    pts = {}

                    def _c0(kb):
                        return max(kb - 4 * qc, 0) * 128

                    for step in range(kbmax + LAG):
                        if step < kbmax:
                            kb, c0 = step, _c0(step)
                            st_t = stp.tile([128, CH], F32, tag="st")
                            nc.tensor.matmul(
                                st_t[:, c0:],
                                kt[:, kb * 128 : (kb + 1) * 128],
                                qt[h][:, qc * CH + c0 : (qc + 1) * CH],
                                start=True,
                                stop=True,
                            )
                            pt = sbw.tile([128, CH], BF, tag="pt", bufs=10)
                            nc.scalar.activation(
                                pt[:, c0:], st_t[:, c0:], EXP,
                                bias=bias_t[:], scale=SCALE,
                            )
                            if kb - 4 * qc >= 0:  # diagonal block
                                nc.vector.tensor_mul(
                                    pt[:, c0 : c0 + 128],
                                    pt[:, c0 : c0 + 128],
                                    tri_sb[:],
                                )
                            pts[kb] = pt
                            _pull(filler, 1)
                        if step >= LAG:
                            kb, c0 = step - LAG, _c0(step - LAG)
                            pt = pts.pop(kb)
                            # columns < c0 are never read: P@V and the
                            # acc accumulation are narrowed to [c0:]
                            nc.tensor.matmul(
                                ot_ps[:, c0:], vsb[:, kb, :], pt[:, c0:],
                                start=(kb == 0), stop=(kb == kbmax - 1),
                            )
                            _pull(filler, 1)
                            if kb == 0:
                                nc.vector.tensor_copy(acc[:], pt[:])
                            else:
                                nc.vector.tensor_add(
                                    acc[:, c0:], acc[:, c0:], pt[:, c0:]
                                )
                    lsum = sbw.tile([128, CH], F32, tag="lsum", bufs=2)
                    nc.gpsimd.partition_all_reduce(
                        lsum[:], acc[:], 128, bass_isa.ReduceOp.add
                    )
                    rlb = sbw.tile([128, CH], F32, tag="rlb", bufs=2)
                    nc.vector.reciprocal_approx_fast(rlb[:], lsum[:])
                    o = otp.tile([128, CH], BF, tag="otsb", name=f"o{qc}_{h}")
                    nc.vector.tensor_mul(o[:], ot_ps[:], rlb[:])
                    ot_sb.append(o)
                    _pull(filler, 2)
                # drain leftover proj/wo work for this segment
                for _ in filler:
                    pass
                prev_ot = ot_sb
            for _ in wo_gen(NCH - 1, prev_ot):
                pass

    with tile.TileContext(nc) as tc:
        for _rep in range(repeat):
            _one(tc)

    nc.compile()
    return nc


def _rope_tables():
    inv = 1.0 / (
        np.float32(ROPE_THETA)
        ** (np.arange(0, HD, 2, dtype=np.float32) / np.float32(HD))
    )
    pos = np.arange(S, dtype=np.float32)
    freqs = np.outer(pos, inv).astype(np.float32)  # [S, 64]
    import ml_dtypes

    c = np.cos(freqs).T  # [64, S]
    s = np.sin(freqs).T
    cosF = np.concatenate([c, c], axis=0)            # [128, S]
    sinF = np.concatenate([-s, s], axis=0)           # sign-folded
    bf16 = ml_dtypes.bfloat16
    return (
        np.ascontiguousarray(cosF).astype(bf16),
        np.ascontiguousarray(sinF).astype(bf16),
    )


def _prep_in_maps(x, wq, wk, wv, wo):
    import ml_dtypes

    bf16 = ml_dtypes.bfloat16
    x = np.asarray(x, dtype=np.float32)
    wq = np.asarray(wq, dtype=np.float32).astype(bf16)
    wk = np.asarray(wk, dtype=np.float32).astype(bf16)
    wv = np.asarray(wv, dtype=np.float32).astype(bf16)
    wo = np.asarray(wo, dtype=np.float32).astype(bf16)

    perm = np.concatenate([np.arange(0, HD, 2), np.arange(1, HD, 2)])
    cosT, sinT = _rope_tables()

    # tri[k, j] = 1 where j >= k (within-block causal triangle)
    msk = (
        np.arange(128)[None, :] >= np.arange(128)[:, None]
    ).astype(bf16)

    def _pmaj(w, nblk):
        # [nblk*128, M] -> [128, nblk, M] partition-major SBUF layout
        return np.ascontiguousarray(
            w.reshape(nblk, 128, w.shape[-1]).transpose(1, 0, 2)
        )

    in_maps = []
    for c in range(N_CORES):
        b, g = divmod(c, NKV)
        qcols = np.concatenate([(g * NHC + h) * HD + perm for h in range(NHC)])
        xTc = x[b].T.astype(bf16)  # [D, S]
        # xP[p, ch, cblk, s'] = xT[cblk*128+p, ch*CH+s']
        xP = np.ascontiguousarray(
            xTc.reshape(NB, 128, NCH, CH).transpose(1, 2, 0, 3)
        )
        # wq head-major: [128, NHC, NB, HD] so each head's slice is one
        # contiguous DMA
        wqp = _pmaj(wq[:, qcols], NB)  # [128, NB, NHC*HD]
        wqp = np.ascontiguousarray(
            wqp.reshape(128, NB, NHC, HD).transpose(0, 2, 1, 3)
        )
        in_maps.append(
            {
                "xT": xP,
                "wq": wqp,
                "wk": _pmaj(wk[:, g * HD + perm], NB),
                "wv": _pmaj(wv[:, g * HD : (g + 1) * HD], NB),
                "wo": _pmaj(wo[g * DQ : (g + 1) * DQ, :], NHC),
                "cosT": cosT,
                "sinT": sinT,
                "msk": msk,
            }
        )
    return in_maps


class _Runner:
    """Build + jit the SPMD program once; reuse across kernel() calls.

    Mirrors bass_utils.run_bass_kernel_spmd's axon path (bass2jax
    run_bass_via_pjrt), but caches the jitted executable so repeated
    kernel() calls don't re-run the multi-minute NEFF compile.
    """

    def __init__(self, repeat=1):
        import jax
        import concourse.mybir as mybir
        from concourse import bass2jax
        from jax.experimental.shard_map import shard_map
        from jax.sharding import Mesh, PartitionSpec

        self.jax = jax
        nc = _build_nc(repeat)
        assert nc.dbg_addr is None
        bass2jax.install_neuronx_cc_hook()

        partition_name = (
            nc.partition_id_tensor.name if nc.partition_id_tensor else None
        )
        in_names, out_names, out_avals, zero_outs = [], [], [], []
        for alloc in nc.m.functions[0].allocations:
            if not isinstance(alloc, mybir.MemoryLocationSet):
                continue
            name = alloc.memorylocations[0].name
            if alloc.kind == "ExternalInput":
                if name != partition_name:
                    in_names.append(name)
            elif alloc.kind == "ExternalOutput":
                shape = tuple(alloc.tensor_shape)
                dtype = mybir.dt.np(alloc.dtype)
                out_names.append(name)
                out_avals.append(jax.core.ShapedArray(shape, dtype))
                zero_outs.append(np.zeros(shape, dtype))
        n_params = len(in_names)
        all_in = in_names + out_names + (
            [partition_name] if partition_name else []
        )

        def _body(*args):
            operands = list(args)
            if partition_name is not None:
                operands.append(bass2jax.partition_id_tensor())
            outs = bass2jax._bass_exec_p.bind(
                *operands,
                out_avals=tuple(out_avals),
                in_names=tuple(all_in),
                out_names=tuple(out_names),
                lowering_input_output_aliases=(),
                sim_require_finite=True,
                sim_require_nnan=True,
                nc=nc,
            )
            return tuple(outs)

        devices = jax.devices()[:N_CORES]
        assert len(devices) == N_CORES
        self.mesh = Mesh(np.asarray(devices), ("core",))
        in_specs = (PartitionSpec("core"),) * (n_params + len(out_names))
        out_specs = (PartitionSpec("core"),) * len(out_names)
        donate = tuple(range(n_params, n_params + len(out_names)))
        self.fn = jax.jit(
            shard_map(
                _body,
                mesh=self.mesh,
                in_specs=in_specs,
                out_specs=out_specs,
                check_rep=False,
            ),
            donate_argnums=donate,
            keep_unused=True,
        )
        self.in_names = in_names
        self.out_names = out_names
        self.out_avals = out_avals

    def concat_inputs(self, in_maps):
        return [
            np.concatenate([np.asarray(m[name]) for m in in_maps], axis=0)
            for name in self.in_names
        ]

    def zero_outputs(self):
        return [
            np.zeros((N_CORES * a.shape[0], *a.shape[1:]), a.dtype)
            for a in self.out_avals
        ]

    def time_iters(self, in_maps, iters=20, warmup=3):
        """Amortized per-call wall time (s) with device-resident inputs.

        Reuses each call's outputs as the next call's donated output
        buffers (the kernel writes every output element, so their
        contents don't matter).
        """
        import time

        jax = self.jax
        from jax.sharding import NamedSharding, PartitionSpec

        sh = NamedSharding(self.mesh, PartitionSpec("core"))
        ins = [jax.device_put(a, sh) for a in self.concat_inputs(in_maps)]
        outs = self.fn(
            *ins, *[jax.device_put(z, sh) for z in self.zero_outputs()]
        )
        for _ in range(warmup):
            outs = self.fn(*ins, *outs)
        jax.block_until_ready(outs)
        t0 = time.perf_counter()
        for _ in range(iters):
            outs = self.fn(*ins, *outs)
        jax.block_until_ready(outs)
        return (time.perf_counter() - t0) / iters

    def run(self, in_maps):
        out_arrs = self.fn(*self.concat_inputs(in_maps), *self.zero_outputs())
        return [
            {
                name: np.asarray(out_arrs[i]).reshape(
                    N_CORES, *self.out_avals[i].shape
                )[c]
                for i, name in enumerate(self.out_names)
            }
            for c in range(N_CORES)
        ]


def _get_runner():
    if "runner" not in _CACHE:
        _CACHE["runner"] = _Runner()
    return _CACHE["runner"]


def kernel(x, wq, wk, wv, wo):
    runner = _get_runner()
    results = runner.run(_prep_in_maps(x, wq, wk, wv, wo))
    y = np.zeros((B, S, D), dtype=np.float32)
    for c in range(N_CORES):
        y[c // NKV] += results[c]["y"].astype(np.float32)
    return y
